# revision 11
# baseline (speedup 1.0000x reference)
"""LinkWeightDecoder Trainium2 kernel.

out[e] = MLP(concat(emb[src[e]], emb[dst[e]])) for 1M edges, sharded
data-parallel over 8 NeuronCores; node table + MLP weights replicated.

Device pipeline per core, per 1024-edge batch:
  - dma_gather (SWDGE) pulls 1024 src rows + 1024 dst rows (512B each,
    full DMA line-rate) into SBUF, edge-major [128, 8, 128]
  - per 512-edge group: PE transposes 128x128 blocks to feature-major
    (float32r, 1.5 cyc/row), DVE copies PSUM->SBUF, then layer 1 runs in
    float32r (1 cyc/row) and layers 2-3 in bf16; ACT fuses bias+relu on
    the PSUM->SBUF copies
  - b3 is folded into an augmented W2/b2/W3 (65th unit == constant 1)
  - layer 3 is column-shaped: lhsT=h2s[:,blk] [65,128] x w3 [65,1] ->
    [128,1] PSUM columns; 64 batches of outputs accumulate in one PSUM
    bank before a single ACT copy + 256KB DMA flush

Edges are bucketed host-side by (src>>15, dst>>15) so each gather call's
int16 local indices stay in range with a per-bucket table base offset.
Buckets are split evenly across cores so all 8 cores share one program.
"""
import math
import numpy as np

import concourse.bass as bass
import concourse.bacc as bacc
import concourse.mybir as mybir
import concourse.tile as tile
from concourse.bass_utils import run_bass_kernel_spmd

N = 100000
D = 128
E = 1000000
H1, H2 = 128, 64
H2E = H2 + 1          # augmented: 65th unit is constant 1.0 (folds b3)
NCORES = 8
RS = 32768            # node range size per int16-indexed table slice
NRANGES = (N + RS - 1) // RS
BATCH = 1024          # edges per dma_gather call (SWDGE ring limit)
GROUP = 512           # edges per matmul chain (PSUM bank free limit)
GPB = BATCH // GROUP  # groups per batch
BLK = GROUP // 128    # 128-edge L3 column blocks per group
P3B = 64              # batches of L3 outputs held in one PSUM bank
P3COLS = P3B * GPB * BLK  # = 512 PSUM columns per flush span

f32 = mybir.dt.float32
f32r = mybir.dt.float32r
bf16 = mybir.dt.bfloat16
i16 = mybir.dt.int16

_AF = mybir.ActivationFunctionType


def _wrap_idx(vals):
    """[BATCH] int16 -> [128, BATCH//16] wrap layout (pos i -> [i%16, i//16],
    replicated 8x down the partitions for the 8 Q7 cores)."""
    w = np.zeros((16, BATCH // 16), np.int16)
    w[np.arange(BATCH) % 16, np.arange(BATCH) // 16] = vals
    return np.tile(w, (8, 1))


def _prepare(inputs):
    """Bucket + shard the edges. Returns (caps, nb, gidx, pos2edge, ranges)."""
    ei = np.asarray(inputs["edge_index"]).astype(np.int64)
    src, dst = ei[0], ei[1]
    bucket = (src >> 15) * NRANGES + (dst >> 15)
    order = np.argsort(bucket, kind="stable")

    counts = np.bincount(bucket, minlength=NRANGES * NRANGES)
    caps = []          # per-bucket per-core capacity (multiple of BATCH)
    bucket_ids = []    # bucket ids with nonzero count, in processing order
    for b in range(NRANGES * NRANGES):
        if counts[b] == 0:
            continue
        per_core = math.ceil(counts[b] / NCORES)
        caps.append(math.ceil(per_core / BATCH) * BATCH)
        bucket_ids.append(b)

    ncap = sum(caps)
    nb = ncap // BATCH

    gidx = np.zeros((NCORES, nb, 128, 2 * (BATCH // 16)), np.int16)
    pos2edge = np.full((NCORES, ncap), -1, np.int64)

    boundaries = np.cumsum(counts)
    for k, b in enumerate(bucket_ids):
        lo = boundaries[b] - counts[b]
        ids_all = order[lo:boundaries[b]]
        splits = np.array_split(ids_all, NCORES)
        cap = caps[k]
        base = sum(caps[:k])
        bs, bd = b // NRANGES, b % NRANGES
        for c in range(NCORES):
            ids = splits[c]
            sloc = np.zeros(cap, np.int16)
            dloc = np.zeros(cap, np.int16)
            sloc[: len(ids)] = (src[ids] - (bs << 15)).astype(np.int16)
            dloc[: len(ids)] = (dst[ids] - (bd << 15)).astype(np.int16)
            pos2edge[c, base: base + len(ids)] = ids
            for t in range(cap // BATCH):
                bi = base // BATCH + t
                sl = slice(t * BATCH, (t + 1) * BATCH)
                gidx[c, bi, :, : BATCH // 16] = _wrap_idx(sloc[sl])
                gidx[c, bi, :, BATCH // 16:] = _wrap_idx(dloc[sl])

    ranges = []  # per batch: (src_base, src_len, dst_base, dst_len)
    for k, b in enumerate(bucket_ids):
        bs, bd = b // NRANGES, b % NRANGES
        sb = bs << 15
        db = bd << 15
        sl = min(RS, N - sb)
        dl = min(RS, N - db)
        ranges += [(sb, sl, db, dl)] * (caps[k] // BATCH)

    return caps, nb, gidx, pos2edge, ranges


def _out_perm(nb):
    """Device out_d is [NF, 128, P3COLS]: value for core position p lives at
    [bi//P3B, p%128, (bi%P3B)*GPB*BLK + g*BLK + blk]. Returns perm such that
    dev_flat[perm[p]] == out value for core position p."""
    p = np.arange(nb * BATCH)
    bi = p // BATCH
    g = (p % BATCH) // GROUP
    blk = (p % GROUP) // 128
    r = p % 128
    f = bi // P3B
    col = (bi % P3B) * (GPB * BLK) + g * BLK + blk
    return f * (128 * P3COLS) + r * P3COLS + col


def _build_program(nb, ranges, b3f, mode="f32r"):
    nc = bacc.Bacc()
    if mode == "f32r":
        gdt = f32r        # gather-path dtype: table, W1, identity, transposes
        mdt = bf16        # layers 2-3 dtype: h1s, h2s, w2, w3
    elif mode == "f32":
        gdt = f32
        mdt = f32
    else:
        gdt = bf16
        mdt = bf16
    table = nc.dram_tensor("table", [N, D], gdt, kind="ExternalInput")
    gidx = nc.dram_tensor("gidx", [nb, 128, 2 * (BATCH // 16)], i16,
                          kind="ExternalInput")
    w1a = nc.dram_tensor("w1a", [D, H1], gdt, kind="ExternalInput")
    w1b = nc.dram_tensor("w1b", [D, H1], gdt, kind="ExternalInput")
    w2 = nc.dram_tensor("w2", [H1, H2E], mdt, kind="ExternalInput")
    w3 = nc.dram_tensor("w3", [H2E, 1], mdt, kind="ExternalInput")
    b1 = nc.dram_tensor("b1", [H1, 1], f32, kind="ExternalInput")
    b2 = nc.dram_tensor("b2", [H2E, 1], f32, kind="ExternalInput")
    ident = nc.dram_tensor("ident", [128, 128], gdt, kind="ExternalInput")
    NF = (nb + P3B - 1) // P3B
    out_d = nc.dram_tensor("out", [NF, 128, P3COLS], f32,
                           kind="ExternalOutput")

    with tile.TileContext(nc) as tc:
        with (
            tc.tile_pool(name="const", bufs=1) as cpool,
            tc.tile_pool(name="gath", bufs=3) as gpool,
            tc.tile_pool(name="idx", bufs=3) as ipool,
            tc.tile_pool(name="tsb", bufs=4) as tpool,
            tc.tile_pool(name="hsb", bufs=3) as hpool,
            tc.tile_pool(name="osb", bufs=2) as opool,
            tc.tile_pool(name="pT", bufs=3, space="PSUM") as pTp,
            tc.tile_pool(name="pH", bufs=2, space="PSUM") as pHp,
            tc.tile_pool(name="p2p", bufs=1, space="PSUM") as p2p,
            tc.tile_pool(name="p3p", bufs=2, space="PSUM") as p3p,
        ):
            w1a_t = cpool.tile([D, H1], gdt)
            w1b_t = cpool.tile([D, H1], gdt)
            w2_t = cpool.tile([H1, H2E], mdt)
            w3_t = cpool.tile([H2E, 1], mdt)
            b1_t = cpool.tile([H1, 1], f32)
            b2_t = cpool.tile([H2E, 1], f32)
            nc.sync.dma_start(out=w1a_t[:], in_=w1a[:, :])
            nc.sync.dma_start(out=w1b_t[:], in_=w1b[:, :])
            nc.sync.dma_start(out=w2_t[:], in_=w2[:, :])
            nc.sync.dma_start(out=w3_t[:], in_=w3[:, :])
            nc.sync.dma_start(out=b1_t[:], in_=b1[:, :])
            nc.sync.dma_start(out=b2_t[:], in_=b2[:, :])
            if mode != "bf16":
                id_t = cpool.tile([128, 128], gdt)
                nc.sync.dma_start(out=id_t[:], in_=ident[:, :])

            p3 = None
            flush_lo = 0
            for bi in range(nb):
                sb_, sl_, db_, dl_ = ranges[bi]
                if p3 is None:
                    p3 = p3p.tile([128, P3COLS], f32, space="PSUM", tag="p3")
                    flush_lo = bi

                it = ipool.tile([128, 2 * (BATCH // 16)], i16, tag="it")
                nc.sync.dma_start(out=it[:], in_=gidx[bi, :, :])

                if mode != "bf16":
                    g_s = gpool.tile([128, (BATCH // 128) * D], gdt, tag="gs")
                    g_d = gpool.tile([128, (BATCH // 128) * D], gdt, tag="gd")
                    nc.gpsimd.dma_gather(
                        out_ap=g_s[:].rearrange("p (j f) -> p j f", f=D),
                        in_ap=table[sb_: sb_ + sl_, :],
                        idxs_ap=it[:, : BATCH // 16],
                        num_idxs=BATCH, num_idxs_reg=BATCH, elem_size=D,
                    )
                    nc.gpsimd.dma_gather(
                        out_ap=g_d[:].rearrange("p (j f) -> p j f", f=D),
                        in_ap=table[db_: db_ + dl_, :],
                        idxs_ap=it[:, BATCH // 16:],
                        num_idxs=BATCH, num_idxs_reg=BATCH, elem_size=D,
                    )
                else:
                    g_s = gpool.tile([128, BATCH], bf16, tag="gs")
                    g_d = gpool.tile([128, BATCH], bf16, tag="gd")
                    nc.gpsimd.dma_gather(
                        out_ap=g_s[:].rearrange("p (j n) -> p j n", j=1),
                        in_ap=table[sb_: sb_ + sl_, :],
                        idxs_ap=it[:, : BATCH // 16],
                        num_idxs=BATCH, num_idxs_reg=BATCH, elem_size=D,
                        transpose=True,
                    )
                    nc.gpsimd.dma_gather(
                        out_ap=g_d[:].rearrange("p (j n) -> p j n", j=1),
                        in_ap=table[db_: db_ + dl_, :],
                        idxs_ap=it[:, BATCH // 16:],
                        num_idxs=BATCH, num_idxs_reg=BATCH, elem_size=D,
                        transpose=True,
                    )

                if mode == "bf16":
                    # batch-wide PSUM tiles (2 banks each) halve the number
                    # of relu ops; h2's bias+relu runs fused on DVE so ACT
                    # only handles h1
                    h1p = pHp.tile([H1, BATCH], f32, space="PSUM", tag="pH")
                    p2 = p2p.tile([H2E, BATCH], f32, space="PSUM", tag="p2")
                    for g in range(GPB):
                        rs = g_s[:, g * GROUP:(g + 1) * GROUP]
                        rd = g_d[:, g * GROUP:(g + 1) * GROUP]
                        h1pg = h1p[:, g * GROUP:(g + 1) * GROUP]
                        nc.tensor.matmul(out=h1pg, lhsT=w1a_t[:], rhs=rs,
                                         start=True, stop=False)
                        nc.tensor.matmul(out=h1pg, lhsT=w1b_t[:], rhs=rd,
                                         start=False, stop=True)
                    h1s = hpool.tile([H1, BATCH], mdt, tag="h1")
                    nc.scalar.activation(h1s[:], h1p[:], _AF.Relu, bias=b1_t[:])
                    for g in range(GPB):
                        nc.tensor.matmul(out=p2[:, g * GROUP:(g + 1) * GROUP],
                                         lhsT=w2_t[:],
                                         rhs=h1s[:, g * GROUP:(g + 1) * GROUP],
                                         start=True, stop=True)
                    h2s = hpool.tile([H2E, BATCH], mdt, tag="h2")
                    nc.vector.tensor_scalar(
                        out=h2s[:], in0=p2[:], scalar1=b2_t[:], scalar2=0.0,
                        op0=mybir.AluOpType.add, op1=mybir.AluOpType.max)
                    colbase = (bi - flush_lo) * (GPB * BLK)
                    for jj in range(GPB * BLK):
                        nc.tensor.matmul(
                            out=p3[:, colbase + jj: colbase + jj + 1],
                            lhsT=h2s[:, jj * 128:(jj + 1) * 128],
                            rhs=w3_t[:], start=True, stop=True,
                        )
                else:
                    for g in range(GPB):
                        pTs = pTp.tile([128, GROUP], gdt, space="PSUM", tag="pT")
                        pTd = pTp.tile([128, GROUP], gdt, space="PSUM", tag="pT")
                        for jj in range(GROUP // 128):
                            blk = g * (GROUP // 128) + jj
                            nc.tensor.transpose(
                                out=pTs[:, jj * 128:(jj + 1) * 128],
                                in_=g_s[:, blk * 128:(blk + 1) * 128],
                                identity=id_t[:],
                            )
                            nc.tensor.transpose(
                                out=pTd[:, jj * 128:(jj + 1) * 128],
                                in_=g_d[:, blk * 128:(blk + 1) * 128],
                                identity=id_t[:],
                            )
                        srcT = tpool.tile([128, GROUP], gdt, tag="tT")
                        dstT = tpool.tile([128, GROUP], gdt, tag="tT")
                        nc.vector.tensor_copy(out=srcT[:], in_=pTs[:])
                        nc.vector.tensor_copy(out=dstT[:], in_=pTd[:])

                        h1p = pHp.tile([128, GROUP], f32, space="PSUM", tag="pH")
                        nc.tensor.matmul(out=h1p[:], lhsT=w1a_t[:], rhs=srcT[:],
                                         start=True, stop=False)
                        nc.tensor.matmul(out=h1p[:], lhsT=w1b_t[:], rhs=dstT[:],
                                         start=False, stop=True)
                        h1s = hpool.tile([H1, GROUP], mdt, tag="h1")
                        nc.scalar.activation(h1s[:], h1p[:], _AF.Relu,
                                             bias=b1_t[:])

                        p2 = p2p.tile([H2E, GROUP], f32, space="PSUM", tag="p2")
                        nc.tensor.matmul(out=p2[:], lhsT=w2_t[:], rhs=h1s[:],
                                         start=True, stop=True)
                        h2s = hpool.tile([H2E, GROUP], mdt, tag="h2")
                        nc.scalar.activation(h2s[:], p2[:], _AF.Relu,
                                             bias=b2_t[:])

                        colbase = (bi - flush_lo) * (GPB * BLK) + g * BLK
                        for jj in range(BLK):
                            nc.tensor.matmul(
                                out=p3[:, colbase + jj: colbase + jj + 1],
                                lhsT=h2s[:, jj * 128:(jj + 1) * 128],
                                rhs=w3_t[:], start=True, stop=True,
                            )

                if bi - flush_lo == P3B - 1 or bi == nb - 1:
                    ncols = (bi - flush_lo + 1) * (GPB * BLK)
                    outsb = opool.tile([128, P3COLS], f32, tag="outsb")
                    nc.scalar.activation(
                        outsb[:, :ncols], p3[:, :ncols], _AF.Copy, bias=0.0,
                    )
                    nc.sync.dma_start(
                        out=out_d[flush_lo // P3B, :, :ncols],
                        in_=outsb[:, :ncols],
                    )
                    p3 = None

    nc.compile()
    return nc


def _in_maps(inputs, gidx, mode):
    import ml_dtypes
    gnp = np.float32 if mode != "bf16" else ml_dtypes.bfloat16
    mnp = np.float32 if mode == "f32" else ml_dtypes.bfloat16
    emb = np.asarray(inputs["node_embeddings"], np.float32)
    W1 = np.asarray(inputs["W1"], np.float32)
    W2 = np.asarray(inputs["W2"], np.float32)
    W3 = np.asarray(inputs["W3"], np.float32)
    b2v = np.asarray(inputs["b2"], np.float32).reshape(H2)
    b3f = float(np.asarray(inputs["b3"], np.float32).reshape(-1)[0])
    # augmentation: h2aug[64] = relu(0*h1 + 1) = 1, w3aug[64] = b3
    w2aug = np.concatenate([W2, np.zeros((H1, 1), np.float32)], axis=1)
    b2aug = np.concatenate([b2v, [1.0]]).astype(np.float32)
    w3aug = np.concatenate([W3, [[b3f]]], axis=0).astype(np.float32)
    maps = []
    base = {
        "table": np.ascontiguousarray(emb.astype(gnp)),
        "w1a": np.ascontiguousarray(W1[:D].astype(gnp)),
        "w1b": np.ascontiguousarray(W1[D:].astype(gnp)),
        "w2": np.ascontiguousarray(w2aug.astype(mnp)),
        "w3": np.ascontiguousarray(w3aug.astype(mnp)),
        "b1": np.asarray(inputs["b1"], np.float32).reshape(H1, 1),
        "b2": b2aug.reshape(H2E, 1),
        "ident": np.eye(128, dtype=gnp),
    }
    for c in range(NCORES):
        m = dict(base)
        m["gidx"] = gidx[c]
        maps.append(m)
    return maps


MODE = "f32r"


def kernel(**inputs):
    mode = MODE
    caps, nb, gidx, pos2edge, ranges = _prepare(inputs)
    b3f = float(np.asarray(inputs["b3"], np.float32).reshape(-1)[0])
    nc = _build_program(nb, ranges, b3f, mode)
    maps = _in_maps(inputs, gidx, mode)
    res = run_bass_kernel_spmd(nc, maps, list(range(NCORES)))

    perm = _out_perm(nb)
    out = np.zeros(E, np.float32)
    for c in range(NCORES):
        dev = res.results[c]["out"].reshape(-1)[perm]
        m = pos2edge[c] >= 0
        out[pos2edge[c][m]] = dev[m]
    return out.reshape(E, 1)


# revision 20
# speedup vs baseline: 1.1612x; 1.1612x over previous
"""LinkWeightDecoder Trainium2 kernel.

out[e] = MLP(concat(emb[src[e]], emb[dst[e]])) for 1M edges, sharded
data-parallel over 8 NeuronCores; node table + MLP weights replicated.

Device pipeline per core, per 1024-edge batch:
  - dma_gather (SWDGE) pulls 1024 src rows + 1024 dst rows (512B each,
    full DMA line-rate) into SBUF, edge-major [128, 8, 128]
  - per 512-edge group: PE transposes 128x128 blocks to feature-major
    (float32r, 1.5 cyc/row), DVE copies PSUM->SBUF, then layer 1 runs in
    float32r (1 cyc/row) and layers 2-3 in bf16; ACT fuses bias+relu on
    the PSUM->SBUF copies
  - b3 is folded into an augmented W2/b2/W3 (65th unit == constant 1)
  - layer 3 is column-shaped: lhsT=h2s[:,blk] [65,128] x w3 [65,1] ->
    [128,1] PSUM columns; 64 batches of outputs accumulate in one PSUM
    bank before a single ACT copy + 256KB DMA flush

Edges are bucketed host-side by (src>>15, dst>>15) so each gather call's
int16 local indices stay in range with a per-bucket table base offset.
Buckets are split evenly across cores so all 8 cores share one program.
"""
import math
import numpy as np

import concourse.bass as bass
import concourse.bacc as bacc
import concourse.mybir as mybir
import concourse.tile as tile
from concourse.bass_utils import run_bass_kernel_spmd

N = 100000
D = 128
E = 1000000
H1, H2 = 128, 64
H2E = H2 + 1          # augmented: 65th unit is constant 1.0 (folds b3)
NCORES = 8
RS = 32768            # node range size per int16-indexed table slice
NRANGES = (N + RS - 1) // RS
BATCH = 1024          # edges per dma_gather call (SWDGE ring limit)
GROUP = 512           # edges per matmul chain (PSUM bank free limit)
GPB = BATCH // GROUP  # groups per batch
BLK = GROUP // 128    # 128-edge L3 column blocks per group
P3B = 64              # batches of L3 outputs held in one PSUM bank
P3COLS = P3B * GPB * BLK  # = 512 PSUM columns per flush span

f32 = mybir.dt.float32
f32r = mybir.dt.float32r
bf16 = mybir.dt.bfloat16
i16 = mybir.dt.int16

_AF = mybir.ActivationFunctionType


def _wrap_idx(vals):
    """[BATCH] int16 -> [128, BATCH//16] wrap layout (pos i -> [i%16, i//16],
    replicated 8x down the partitions for the 8 Q7 cores)."""
    w = np.zeros((16, BATCH // 16), np.int16)
    w[np.arange(BATCH) % 16, np.arange(BATCH) // 16] = vals
    return np.tile(w, (8, 1))


def _prepare(inputs):
    """Bucket + shard the edges. Returns (caps, nb, gidx, pos2edge, ranges)."""
    ei = np.asarray(inputs["edge_index"]).astype(np.int64)
    src, dst = ei[0], ei[1]
    bucket = (src >> 15) * NRANGES + (dst >> 15)
    order = np.argsort(bucket, kind="stable")

    counts = np.bincount(bucket, minlength=NRANGES * NRANGES)
    caps = []          # per-bucket per-core capacity (multiple of BATCH)
    bucket_ids = []    # bucket ids with nonzero count, in processing order
    for b in range(NRANGES * NRANGES):
        if counts[b] == 0:
            continue
        per_core = math.ceil(counts[b] / NCORES)
        caps.append(math.ceil(per_core / BATCH) * BATCH)
        bucket_ids.append(b)

    ncap = sum(caps)
    nb = ncap // BATCH

    gidx = np.zeros((NCORES, nb, 128, 2 * (BATCH // 16)), np.int16)
    pos2edge = np.full((NCORES, ncap), -1, np.int64)

    boundaries = np.cumsum(counts)
    for k, b in enumerate(bucket_ids):
        lo = boundaries[b] - counts[b]
        ids_all = order[lo:boundaries[b]]
        splits = np.array_split(ids_all, NCORES)
        cap = caps[k]
        base = sum(caps[:k])
        bs, bd = b // NRANGES, b % NRANGES
        for c in range(NCORES):
            ids = splits[c]
            # -1 padding: SWDGE skips trailing negative indices, so padded
            # tail positions cost no DMA descriptors (non-transpose mode)
            sloc = np.full(cap, -1, np.int16)
            dloc = np.full(cap, -1, np.int16)
            sloc[: len(ids)] = (src[ids] - (bs << 15)).astype(np.int16)
            dloc[: len(ids)] = (dst[ids] - (bd << 15)).astype(np.int16)
            pos2edge[c, base: base + len(ids)] = ids
            for t in range(cap // BATCH):
                bi = base // BATCH + t
                sl = slice(t * BATCH, (t + 1) * BATCH)
                gidx[c, bi, :, : BATCH // 16] = _wrap_idx(sloc[sl])
                gidx[c, bi, :, BATCH // 16:] = _wrap_idx(dloc[sl])

    ranges = []  # per batch: (src_base, src_len, dst_base, dst_len)
    for k, b in enumerate(bucket_ids):
        bs, bd = b // NRANGES, b % NRANGES
        sb = bs << 15
        db = bd << 15
        sl = min(RS, N - sb)
        dl = min(RS, N - db)
        ranges += [(sb, sl, db, dl)] * (caps[k] // BATCH)

    return caps, nb, gidx, pos2edge, ranges


def _out_perm(nb):
    """Device out_d is [NF, 128, P3COLS]: value for core position p lives at
    [bi//P3B, p%128, (bi%P3B)*GPB*BLK + g*BLK + blk]. Returns perm such that
    dev_flat[perm[p]] == out value for core position p."""
    p = np.arange(nb * BATCH)
    bi = p // BATCH
    g = (p % BATCH) // GROUP
    blk = (p % GROUP) // 128
    r = p % 128
    f = bi // P3B
    col = (bi % P3B) * (GPB * BLK) + g * BLK + blk
    return f * (128 * P3COLS) + r * P3COLS + col


NQ = 4                # SWDGE queues; gathers rotate across them


def _build_program(nb, ranges, b3f, mode="f32r", variant="full"):
    nc = bacc.Bacc(num_swdge_queues=NQ)
    if mode == "f32r":
        gdt = f32r        # gather-path dtype: table, W1, identity, transposes
        mdt = bf16        # layers 2-3 dtype: h1s, h2s, w2, w3
    elif mode == "f32":
        gdt = f32
        mdt = f32
    else:
        gdt = bf16
        mdt = bf16
    table = nc.dram_tensor("table", [N, D], gdt, kind="ExternalInput")
    gidx = nc.dram_tensor("gidx", [nb, 128, 2 * (BATCH // 16)], i16,
                          kind="ExternalInput")
    w1a = nc.dram_tensor("w1a", [D, H1], gdt, kind="ExternalInput")
    w1b = nc.dram_tensor("w1b", [D, H1], gdt, kind="ExternalInput")
    w2 = nc.dram_tensor("w2", [H1, H2E], mdt, kind="ExternalInput")
    w3 = nc.dram_tensor("w3", [H2E, 1], mdt, kind="ExternalInput")
    b1 = nc.dram_tensor("b1", [H1, 1], f32, kind="ExternalInput")
    b2 = nc.dram_tensor("b2", [H2E, 1], f32, kind="ExternalInput")
    ident = nc.dram_tensor("ident", [128, 128], gdt, kind="ExternalInput")
    NF = (nb + P3B - 1) // P3B
    out_d = nc.dram_tensor("out", [NF, 128, P3COLS], f32,
                           kind="ExternalOutput")

    with tile.TileContext(nc) as tc:
        with (
            tc.tile_pool(name="const", bufs=1) as cpool,
            tc.tile_pool(name="gath", bufs=3) as gpool,
            tc.tile_pool(name="idx", bufs=3) as ipool,
            tc.tile_pool(name="tsb", bufs=4) as tpool,
            tc.tile_pool(name="hsb", bufs=3) as hpool,
            tc.tile_pool(name="osb", bufs=2) as opool,
            tc.tile_pool(name="pT", bufs=3, space="PSUM") as pTp,
            tc.tile_pool(name="pH", bufs=2, space="PSUM") as pHp,
            tc.tile_pool(name="p2p", bufs=1, space="PSUM") as p2p,
            tc.tile_pool(name="p3p", bufs=2, space="PSUM") as p3p,
        ):
            w1a_t = cpool.tile([D, H1], gdt)
            w1b_t = cpool.tile([D, H1], gdt)
            w2_t = cpool.tile([H1, H2E], mdt)
            w3_t = cpool.tile([H2E, 1], mdt)
            b1_t = cpool.tile([H1, 1], f32)
            b2_t = cpool.tile([H2E, 1], f32)
            nc.sync.dma_start(out=w1a_t[:], in_=w1a[:, :])
            nc.sync.dma_start(out=w1b_t[:], in_=w1b[:, :])
            nc.sync.dma_start(out=w2_t[:], in_=w2[:, :])
            nc.sync.dma_start(out=w3_t[:], in_=w3[:, :])
            nc.sync.dma_start(out=b1_t[:], in_=b1[:, :])
            nc.sync.dma_start(out=b2_t[:], in_=b2[:, :])
            if mode != "bf16":
                id_t = cpool.tile([128, 128], gdt)
                nc.sync.dma_start(out=id_t[:], in_=ident[:, :])

            if variant == "nogather":
                gw = (BATCH // 128) * D if mode != "bf16" else BATCH
                g_fix = cpool.tile([128, gw], gdt if mode != "bf16" else bf16)
                # walrus rejects Memset on float32r; set the bits as f32
                nc.vector.memset(g_fix[:].bitcast(f32) if gdt == f32r
                                 else g_fix[:], 0.0)

            p3 = None
            flush_lo = 0
            for bi in range(nb):
                sb_, sl_, db_, dl_ = ranges[bi]
                if p3 is None:
                    p3 = p3p.tile([128, P3COLS], f32, space="PSUM", tag="p3")
                    flush_lo = bi

                it = ipool.tile([128, 2 * (BATCH // 16)], i16, tag="it")
                nc.sync.dma_start(out=it[:], in_=gidx[bi, :, :])

                if variant == "nogather":
                    g_s, g_d = g_fix, g_fix
                elif mode != "bf16":
                    g_s = gpool.tile([128, (BATCH // 128) * D], gdt, tag="gs")
                    g_d = gpool.tile([128, (BATCH // 128) * D], gdt, tag="gd")
                    nc.gpsimd.dma_gather(
                        out_ap=g_s[:].rearrange("p (j f) -> p j f", f=D),
                        in_ap=table[sb_: sb_ + sl_, :],
                        idxs_ap=it[:, : BATCH // 16],
                        num_idxs=BATCH, num_idxs_reg=BATCH, elem_size=D,
                        queue_num=(2 * bi) % NQ,
                    )
                    nc.gpsimd.dma_gather(
                        out_ap=g_d[:].rearrange("p (j f) -> p j f", f=D),
                        in_ap=table[db_: db_ + dl_, :],
                        idxs_ap=it[:, BATCH // 16:],
                        num_idxs=BATCH, num_idxs_reg=BATCH, elem_size=D,
                        queue_num=(2 * bi + 1) % NQ,
                    )
                else:
                    g_s = gpool.tile([128, BATCH], bf16, tag="gs")
                    g_d = gpool.tile([128, BATCH], bf16, tag="gd")
                    nc.gpsimd.dma_gather(
                        out_ap=g_s[:].rearrange("p (j n) -> p j n", j=1),
                        in_ap=table[sb_: sb_ + sl_, :],
                        idxs_ap=it[:, : BATCH // 16],
                        num_idxs=BATCH, num_idxs_reg=BATCH, elem_size=D,
                        transpose=True, queue_num=(2 * bi) % NQ,
                    )
                    nc.gpsimd.dma_gather(
                        out_ap=g_d[:].rearrange("p (j n) -> p j n", j=1),
                        in_ap=table[db_: db_ + dl_, :],
                        idxs_ap=it[:, BATCH // 16:],
                        num_idxs=BATCH, num_idxs_reg=BATCH, elem_size=D,
                        transpose=True, queue_num=(2 * bi + 1) % NQ,
                    )

                if variant == "gatheronly":
                    continue
                if mode == "bf16":
                    # batch-wide PSUM tiles (2 banks each) halve the number
                    # of relu ops; h2's bias+relu runs fused on DVE so ACT
                    # only handles h1
                    h1p = pHp.tile([H1, BATCH], f32, space="PSUM", tag="pH")
                    p2 = p2p.tile([H2E, BATCH], f32, space="PSUM", tag="p2")
                    for g in range(GPB):
                        rs = g_s[:, g * GROUP:(g + 1) * GROUP]
                        rd = g_d[:, g * GROUP:(g + 1) * GROUP]
                        h1pg = h1p[:, g * GROUP:(g + 1) * GROUP]
                        nc.tensor.matmul(out=h1pg, lhsT=w1a_t[:], rhs=rs,
                                         start=True, stop=False)
                        nc.tensor.matmul(out=h1pg, lhsT=w1b_t[:], rhs=rd,
                                         start=False, stop=True)
                    h1s = hpool.tile([H1, BATCH], mdt, tag="h1")
                    nc.scalar.activation(h1s[:], h1p[:], _AF.Relu, bias=b1_t[:])
                    for g in range(GPB):
                        nc.tensor.matmul(out=p2[:, g * GROUP:(g + 1) * GROUP],
                                         lhsT=w2_t[:],
                                         rhs=h1s[:, g * GROUP:(g + 1) * GROUP],
                                         start=True, stop=True)
                    h2s = hpool.tile([H2E, BATCH], mdt, tag="h2")
                    nc.vector.tensor_scalar(
                        out=h2s[:], in0=p2[:], scalar1=b2_t[:], scalar2=0.0,
                        op0=mybir.AluOpType.add, op1=mybir.AluOpType.max)
                    colbase = (bi - flush_lo) * (GPB * BLK)
                    for jj in range(GPB * BLK):
                        nc.tensor.matmul(
                            out=p3[:, colbase + jj: colbase + jj + 1],
                            lhsT=h2s[:, jj * 128:(jj + 1) * 128],
                            rhs=w3_t[:], start=True, stop=True,
                        )
                else:
                    for g in range(GPB):
                        pTs = pTp.tile([128, GROUP], gdt, space="PSUM", tag="pT")
                        pTd = pTp.tile([128, GROUP], gdt, space="PSUM", tag="pT")
                        for jj in range(GROUP // 128):
                            blk = g * (GROUP // 128) + jj
                            nc.tensor.transpose(
                                out=pTs[:, jj * 128:(jj + 1) * 128],
                                in_=g_s[:, blk * 128:(blk + 1) * 128],
                                identity=id_t[:],
                            )
                            nc.tensor.transpose(
                                out=pTd[:, jj * 128:(jj + 1) * 128],
                                in_=g_d[:, blk * 128:(blk + 1) * 128],
                                identity=id_t[:],
                            )
                        srcT = tpool.tile([128, GROUP], gdt, tag="tT")
                        dstT = tpool.tile([128, GROUP], gdt, tag="tT")
                        nc.vector.tensor_copy(out=srcT[:], in_=pTs[:])
                        nc.vector.tensor_copy(out=dstT[:], in_=pTd[:])

                        h1p = pHp.tile([128, GROUP], f32, space="PSUM", tag="pH")
                        nc.tensor.matmul(out=h1p[:], lhsT=w1a_t[:], rhs=srcT[:],
                                         start=True, stop=False)
                        nc.tensor.matmul(out=h1p[:], lhsT=w1b_t[:], rhs=dstT[:],
                                         start=False, stop=True)
                        h1s = hpool.tile([H1, GROUP], mdt, tag="h1")
                        nc.scalar.activation(h1s[:], h1p[:], _AF.Relu,
                                             bias=b1_t[:])

                        p2 = p2p.tile([H2E, GROUP], f32, space="PSUM", tag="p2")
                        nc.tensor.matmul(out=p2[:], lhsT=w2_t[:], rhs=h1s[:],
                                         start=True, stop=True)
                        h2s = hpool.tile([H2E, GROUP], mdt, tag="h2")
                        nc.scalar.activation(h2s[:], p2[:], _AF.Relu,
                                             bias=b2_t[:])

                        colbase = (bi - flush_lo) * (GPB * BLK) + g * BLK
                        for jj in range(BLK):
                            nc.tensor.matmul(
                                out=p3[:, colbase + jj: colbase + jj + 1],
                                lhsT=h2s[:, jj * 128:(jj + 1) * 128],
                                rhs=w3_t[:], start=True, stop=True,
                            )

                if bi - flush_lo == P3B - 1 or bi == nb - 1:
                    ncols = (bi - flush_lo + 1) * (GPB * BLK)
                    outsb = opool.tile([128, P3COLS], f32, tag="outsb")
                    nc.scalar.activation(
                        outsb[:, :ncols], p3[:, :ncols], _AF.Copy, bias=0.0,
                    )
                    nc.sync.dma_start(
                        out=out_d[flush_lo // P3B, :, :ncols],
                        in_=outsb[:, :ncols],
                    )
                    p3 = None

    nc.compile()
    return nc


def _in_maps(inputs, gidx, mode):
    import ml_dtypes
    gnp = np.float32 if mode != "bf16" else ml_dtypes.bfloat16
    mnp = np.float32 if mode == "f32" else ml_dtypes.bfloat16
    emb = np.asarray(inputs["node_embeddings"], np.float32)
    W1 = np.asarray(inputs["W1"], np.float32)
    W2 = np.asarray(inputs["W2"], np.float32)
    W3 = np.asarray(inputs["W3"], np.float32)
    b2v = np.asarray(inputs["b2"], np.float32).reshape(H2)
    b3f = float(np.asarray(inputs["b3"], np.float32).reshape(-1)[0])
    # augmentation: h2aug[64] = relu(0*h1 + 1) = 1, w3aug[64] = b3
    w2aug = np.concatenate([W2, np.zeros((H1, 1), np.float32)], axis=1)
    b2aug = np.concatenate([b2v, [1.0]]).astype(np.float32)
    w3aug = np.concatenate([W3, [[b3f]]], axis=0).astype(np.float32)
    maps = []
    base = {
        "table": np.ascontiguousarray(emb.astype(gnp)),
        "w1a": np.ascontiguousarray(W1[:D].astype(gnp)),
        "w1b": np.ascontiguousarray(W1[D:].astype(gnp)),
        "w2": np.ascontiguousarray(w2aug.astype(mnp)),
        "w3": np.ascontiguousarray(w3aug.astype(mnp)),
        "b1": np.asarray(inputs["b1"], np.float32).reshape(H1, 1),
        "b2": b2aug.reshape(H2E, 1),
        "ident": np.eye(128, dtype=gnp),
    }
    for c in range(NCORES):
        m = dict(base)
        m["gidx"] = gidx[c]
        maps.append(m)
    return maps


MODE = "f32r"


def kernel(**inputs):
    mode = MODE
    caps, nb, gidx, pos2edge, ranges = _prepare(inputs)
    b3f = float(np.asarray(inputs["b3"], np.float32).reshape(-1)[0])
    nc = _build_program(nb, ranges, b3f, mode)
    maps = _in_maps(inputs, gidx, mode)
    res = run_bass_kernel_spmd(nc, maps, list(range(NCORES)))

    perm = _out_perm(nb)
    out = np.zeros(E, np.float32)
    for c in range(NCORES):
        dev = res.results[c]["out"].reshape(-1)[perm]
        m = pos2edge[c] >= 0
        out[pos2edge[c][m]] = dev[m]
    return out.reshape(E, 1)
